# revision 1
# baseline (speedup 1.0000x reference)
"""CrossDomainGAT Trainium2 kernel.

Strategy (graph/data parallel, per sharding hint):
  - Destination nodes sharded across 8 cores (6250 dests/core, padded to 6272 =
    49 blocks x 128). Edges routed to the core owning the destination, so the
    per-edge softmax (over heads -- edge-local) and the scatter-add stay local.
  - Each core computes the full Q/V projections for all N source nodes itself
    (weights replicated; cheap on PE) and writes interleaved Q|V rows (bf16,
    512B/row) to a private HBM buffer.
  - Per dest-block of 128 nodes, edges are laid out dest-major: round r holds
    the r-th in-edge of each of the 128 dests (host pads per-dest edge lists;
    dests are degree-sorted so blocks have uniform round counts).  Q|V rows for
    a whole block are fetched with one batched dma_gather (512B elements at
    full DMA bandwidth).  K for the block is computed on the fly and stays
    partition-aligned with the dests, so no K gather is needed.
  - Per-edge math runs dest-major on DVE/ACT; the scatter-add accumulation runs
    on the TensorEngine as identity-matmul accumulation into PSUM.
  - Output projection + residual + LayerNorm fused per block; host un-permutes.

int16 gather indices only reach 32768 rows, so source rows are split into a
"lo" range (rq < LO_CUT) and a "hi" range; each dest's edges are grouped into
lo-rounds then hi-rounds and fetched with two gathers from two base offsets.
"""

import math
import numpy as np
import ml_dtypes

# ---------------------------------------------------------------- problem cfg
D = 128
H = 8
HD = 16
ALPHA = 0.2
LN_EPS = 1e-5

FULL_CFG = dict(
    N=50000,
    E=800000,
    NC=8,          # cores
    CHUNK=512,     # phase-1 node chunk
    RCHUNK=16,     # rounds per compute chunk
)


def _rq_map(n, chunk=512):
    """HBM row order for QV rows: within each phase-1 chunk of 512 nodes the
    SBUF tile [128p, 4j, 256] is written flat, so node n = s + j*128 + p lands
    at row s + p*4 + j.  Pure index math."""
    s = (n // chunk) * chunk
    t = n % chunk
    return s + (t % 128) * 4 + (t // 128)


def host_prep(x, edge_index, edge_attr, cfg):
    """Build per-core input arrays + uniform (SPMD) block metadata."""
    N, E, NC = cfg["N"], cfg["E"], cfg["NC"]
    CHUNK = cfg["CHUNK"]
    DPC = N // NC                      # dests per core
    NB = (DPC + 127) // 128            # dest blocks per core
    DPAD = NB * 128
    NCH = (N + CHUNK - 1) // CHUNK     # phase-1 chunks
    if NCH * CHUNK == N:
        NCH += 1                       # ensure zero slots for the hi sentinel
    NPAD = NCH * CHUNK                 # padded node slots

    row = np.asarray(edge_index[0], dtype=np.int64)
    col = np.asarray(edge_index[1], dtype=np.int64)
    ea = np.asarray(edge_attr, dtype=np.float32)
    x = np.asarray(x, dtype=np.float32)

    # rq row mapping for all source nodes
    rq_all = _rq_map(np.arange(NPAD, dtype=np.int64), CHUNK)
    # gather windows (int16 idx reaches 32768 rows; windows may overlap):
    #   QV rows: 0 = zero row, then row 1 + rq for rq in [0, NPAD)
    #   lo window = rows [0, LO_MAX+1]:       idx = 1 + rq, sentinel 0 (zero row)
    #   hi window = rows [1+HI_BASE, 1+NPAD): idx = rq - HI_BASE, sentinel = a
    #     zero node slot (n >= N) inside the hi window.
    LO_MAX = min(32766, NPAD - 1)      # max rq reachable via the lo window
    HI_BASE = max(0, NPAD - 32768)     # min rq reachable via the hi window
    assert NPAD > N, "need at least one padded node slot for the hi sentinel"
    zero_slots = np.arange(N, NPAD, dtype=np.int64)
    sent_rq = int(rq_all[zero_slots].max())
    assert sent_rq >= HI_BASE
    SENT_HI = sent_rq - HI_BASE
    assert 0 <= SENT_HI <= 32767
    HI_ROWS = NPAD - HI_BASE           # size of hi src window
    assert HI_ROWS <= 32768

    rq_e = rq_all[row]                 # rq of each edge's source
    # category: 0 = lo-only (rq < HI_BASE), 1 = flexible, 2 = hi-only
    cat = np.where(rq_e < HI_BASE, 0, np.where(rq_e <= LO_MAX, 1, 2)).astype(np.int8)
    idx_lo_e = (1 + rq_e).astype(np.int64)          # valid when cat <= 1
    idx_hi_e = (rq_e - HI_BASE).astype(np.int64)    # valid when cat >= 1

    core = col // DPC
    cl = col - core * DPC              # local dest id

    # ---- per-core degree sort + balanced lo/hi assignment
    perms = []
    RLc = np.zeros((NC, NB), dtype=np.int64)
    RHc = np.zeros((NC, NB), dtype=np.int64)
    per_core = []
    for c in range(NC):
        m = core == c
        clc = cl[m]
        catc = cat[m]
        Ld = np.bincount(clc[catc == 0], minlength=DPC)
        Fd = np.bincount(clc[catc == 1], minlength=DPC)
        Hd = np.bincount(clc[catc == 2], minlength=DPC)
        deg = Ld + Fd + Hd
        # balanced per-dest lo count: as close to deg/2 as the windows allow
        lo_n = np.clip((deg + 1) // 2, Ld, Ld + Fd)
        hi_n = deg - lo_n
        # group dests so blocks are homogeneous in (lo_n, hi_n): the per-block
        # round count is max(lo_n) + max(hi_n) over the block
        order = np.lexsort((hi_n, lo_n))
        perm = np.concatenate([order, np.full(DPAD - DPC, -1, np.int64)])
        inv = np.empty(DPC, dtype=np.int64)
        inv[order] = np.arange(DPC)
        perms.append(perm)
        q = inv[clc]                                # dest slot of each edge
        b = q // 128
        lo_s = np.concatenate([lo_n[order], np.zeros(DPAD - DPC, np.int64)])
        hi_s = np.concatenate([hi_n[order], np.zeros(DPAD - DPC, np.int64)])
        RLc[c] = lo_s.reshape(NB, 128).max(axis=1)
        RHc[c] = hi_s.reshape(NB, 128).max(axis=1)
        per_core.append(dict(m=m, q=q, b=b, catc=catc, lo_n=lo_n, clc=clc))

    RL = RLc.max(axis=0)               # uniform across cores
    RH = RHc.max(axis=0)
    RT = RL + RH
    n_rounds = int(RT.sum())

    # ---- per-core slot assignment + packed arrays
    # idx stream per block: lo rounds then hi rounds; stream position
    # i = r*128 + p; wrapped to [16, i//16] then tiled to 128 partitions.
    ea_off = np.concatenate([[0], np.cumsum(RT)])       # eattr col offsets (rounds)
    lo_off = np.concatenate([[0], np.cumsum(RL)])       # idx col offsets (rounds)
    hi_off = np.concatenate([[0], np.cumsum(RH)])
    tot_lo = int(RL.sum())
    tot_hi = int(RH.sum())

    idx_lo_arrs, idx_hi_arrs, ea_arrs, xd_arrs, xtd_arrs = [], [], [], [], []
    for c in range(NC):
        pc = per_core[c]
        m, q, b = pc["m"], pc["q"], pc["b"]
        p = q % 128
        # rank within dest, with lo-only edges first, then flexible, then
        # hi-only: the first lo_n[dest] edges go to the lo pass.
        key = q * 4 + pc["catc"].astype(np.int64)
        sort = np.argsort(key, kind="stable")
        qs = q[sort]
        starts = np.r_[0, np.flatnonzero(np.diff(qs)) + 1]
        counts = np.diff(np.r_[starts, len(qs)])
        rank_sorted = np.arange(len(qs)) - np.repeat(starts, counts)
        rank = np.empty(len(qs), np.int64)
        rank[sort] = rank_sorted
        lo_n_e = pc["lo_n"][pc["clc"]]             # per-edge lo split point
        il = rank < lo_n_e

        # lo slots
        strm_lo = np.zeros(tot_lo * 128, dtype=np.int16)   # sentinel 0
        el = np.flatnonzero(il)
        pos = (lo_off[b[el]] + rank[el]) * 128 + p[el]
        strm_lo[pos] = idx_lo_e[m][el].astype(np.int16)
        # hi slots
        strm_hi = np.full(tot_hi * 128, SENT_HI, dtype=np.int16)
        eh = np.flatnonzero(~il)
        posh = (hi_off[b[eh]] + (rank[eh] - lo_n_e[eh])) * 128 + p[eh]
        strm_hi[posh] = idx_hi_e[m][eh].astype(np.int16)
        # wrap to [16, cols] then tile to 128 partitions
        wl = strm_lo.reshape(-1, 16).T.copy()
        wh = strm_hi.reshape(-1, 16).T.copy()
        idx_lo_arrs.append(np.tile(wl, (8, 1)))
        idx_hi_arrs.append(np.tile(wh, (8, 1)))

        # eattr slot layout: [128 p, rounds, 16] with per-block lo rounds then
        # hi rounds at absolute round = ea_off[b] + r(lo) or + RL[b] + r(hi)
        eac = np.zeros((128, int(RT.sum()), 16), dtype=np.float32)
        r_abs = np.empty(len(qs), np.int64)
        r_abs[el] = ea_off[b[el]] + rank[el]
        r_abs[eh] = ea_off[b[eh]] + RL[b[eh]] + (rank[eh] - lo_n_e[eh])
        eac[p, r_abs] = ea[m]
        ea_arrs.append(eac.reshape(128, -1))

        # dest-side x (residual) and xT (K build), permuted to slot order
        perm = perms[c]
        xd = np.zeros((DPAD, D), dtype=np.float32)
        valid = perm >= 0
        xd[valid] = x[c * DPC + perm[valid]]
        xd_arrs.append(xd)
        xtd_arrs.append(np.ascontiguousarray(xd.T).astype(ml_dtypes.bfloat16))

    # xT for phase 1 (replicated)
    xpad = np.zeros((NPAD, D), dtype=np.float32)
    xpad[:N] = x
    xT = np.ascontiguousarray(xpad.T).astype(ml_dtypes.bfloat16)

    meta = dict(
        cfg=cfg, DPC=DPC, NB=NB, DPAD=DPAD, NCH=NCH, NPAD=NPAD,
        RL=RL.astype(int).tolist(), RH=RH.astype(int).tolist(),
        SENT_HI=SENT_HI, HI_ROWS=HI_ROWS, LO_MAX=LO_MAX, HI_BASE=HI_BASE,
        tot_lo=tot_lo, tot_hi=tot_hi, n_rounds=n_rounds,
        lo_off=lo_off.astype(int).tolist(), hi_off=hi_off.astype(int).tolist(),
        ea_off=ea_off.astype(int).tolist(),
    )
    arrs = dict(
        xT=xT, idx_lo=idx_lo_arrs, idx_hi=idx_hi_arrs, ea=ea_arrs,
        xd=xd_arrs, xtd=xtd_arrs, perms=perms,
    )
    return meta, arrs


# ------------------------------------------------------------------ weights
def host_weights(Wq, Wk, Wv, Wo, bo, gamma, beta):
    bf = ml_dtypes.bfloat16
    t = lambda W: np.ascontiguousarray(np.asarray(W, np.float32).T).astype(bf)
    rep = lambda v: np.tile(np.asarray(v, np.float32)[None, :], (128, 1))
    return dict(
        wq_t=t(Wq), wk_t=t(Wk), wv_t=t(Wv), wo_t=t(Wo),
        bo_b=rep(bo), gamma_b=rep(gamma), beta_b=rep(beta),
        ident=np.eye(128, dtype=np.float32).astype(bf),
    )


# ------------------------------------------------------------------ kernel IR
def build_nc(meta, debug=False, stage=None):
    import os as _os
    stage = stage or _os.environ.get("K_STAGE", "full")
    from contextlib import ExitStack
    import concourse.bacc as bacc
    import concourse.bass as bass
    import concourse.tile as tile
    from concourse import mybir

    cfg = meta["cfg"]
    NB, DPAD, NCH, NPAD = meta["NB"], meta["DPAD"], meta["NCH"], meta["NPAD"]
    RL, RH = meta["RL"], meta["RH"]
    CHUNK = cfg["CHUNK"]
    RCHUNK = cfg["RCHUNK"]
    LO_MAX, HI_BASE = meta["LO_MAX"], meta["HI_BASE"]
    HI_ROWS = meta["HI_ROWS"]
    tot_lo, tot_hi = meta["tot_lo"], meta["tot_hi"]
    n_rounds = meta["n_rounds"]
    lo_off, hi_off, ea_off = meta["lo_off"], meta["hi_off"], meta["ea_off"]
    RLMAX, RHMAX = max(RL), max(max(RH), 1)

    dt = mybir.dt
    AF = mybir.ActivationFunctionType
    AL = mybir.AluOpType

    nc = bacc.Bacc("TRN2", target_bir_lowering=False, debug=debug,
                   num_swdge_queues=4)

    # ---------- I/O ----------
    xT_d = nc.dram_tensor("xT", [128, NPAD], dt.bfloat16, kind="ExternalInput")
    xtd_d = nc.dram_tensor("xtd", [128, DPAD], dt.bfloat16, kind="ExternalInput")
    xd_d = nc.dram_tensor("xd", [DPAD, 128], dt.float32, kind="ExternalInput")
    idxlo_d = nc.dram_tensor("idx_lo", [128, tot_lo * 8], dt.int16, kind="ExternalInput")
    idxhi_d = nc.dram_tensor("idx_hi", [128, tot_hi * 8], dt.int16, kind="ExternalInput")
    ea_d = nc.dram_tensor("ea", [128, n_rounds * 16], dt.float32, kind="ExternalInput")
    wq_d = nc.dram_tensor("wq_t", [128, 128], dt.bfloat16, kind="ExternalInput")
    wk_d = nc.dram_tensor("wk_t", [128, 128], dt.bfloat16, kind="ExternalInput")
    wv_d = nc.dram_tensor("wv_t", [128, 128], dt.bfloat16, kind="ExternalInput")
    wo_d = nc.dram_tensor("wo_t", [128, 128], dt.bfloat16, kind="ExternalInput")
    bo_d = nc.dram_tensor("bo_b", [128, 128], dt.float32, kind="ExternalInput")
    ga_d = nc.dram_tensor("gamma_b", [128, 128], dt.float32, kind="ExternalInput")
    be_d = nc.dram_tensor("beta_b", [128, 128], dt.float32, kind="ExternalInput")
    id_d = nc.dram_tensor("ident", [128, 128], dt.bfloat16, kind="ExternalInput")
    y_d = nc.dram_tensor("y", [DPAD, 128], dt.float32, kind="ExternalOutput")

    # private HBM buffer of interleaved Q|V rows (bf16): row 0 zero, 1+rq
    qv_d = nc.dram_tensor("qv", [1 + NPAD, 256], dt.bfloat16)

    JC = CHUNK // 128  # sub-matmuls per phase-1 chunk

    with tile.TileContext(nc) as tc, ExitStack() as ctx:
        consts = ctx.enter_context(tc.tile_pool(name="consts", bufs=1))
        gpool = ctx.enter_context(tc.tile_pool(name="gath", bufs=2))
        mpool = ctx.enter_context(tc.tile_pool(name="meta", bufs=2))
        cpool = ctx.enter_context(tc.tile_pool(name="comp", bufs=3))
        spool = ctx.enter_context(tc.tile_pool(name="small", bufs=4))
        kpool = ctx.enter_context(tc.tile_pool(name="kblk", bufs=2))
        opool = ctx.enter_context(tc.tile_pool(name="outs", bufs=3))

        # ---------- constants ----------
        wq = consts.tile([128, 128], dt.bfloat16)
        wk = consts.tile([128, 128], dt.bfloat16)
        wvt = consts.tile([128, 128], dt.bfloat16)
        wo = consts.tile([128, 128], dt.bfloat16)
        bo = consts.tile([128, 128], dt.float32)
        ga = consts.tile([128, 128], dt.float32)
        be = consts.tile([128, 128], dt.float32)
        ident = consts.tile([128, 128], dt.bfloat16)
        epsT = consts.tile([128, 1], dt.float32)
        zrow = consts.tile([1, 256], dt.bfloat16)
        for dst, src in ((wq, wq_d), (wk, wk_d), (wvt, wv_d), (wo, wo_d),
                         (bo, bo_d), (ga, ga_d), (be, be_d), (ident, id_d)):
            nc.sync.dma_start(out=dst[:], in_=src[:])
        nc.vector.memset(epsT[:], LN_EPS)
        nc.vector.memset(zrow[:], 0.0)
        nc.sync.dma_start(out=qv_d[0:1, :], in_=zrow[:])

        # ---------- phase 1: Q|V rows to HBM ----------
        with tc.tile_pool(name="p1", bufs=3) as p1, \
             tc.tile_pool(name="p1ps", bufs=2, space="PSUM") as p1ps:
            for t in range(NCH):
                xt = p1.tile([128, CHUNK], dt.bfloat16, tag="xt")
                nc.sync.dma_start(out=xt[:], in_=xT_d[:, t * CHUNK:(t + 1) * CHUNK])
                psq = p1ps.tile([128, JC, 128], dt.float32, tag="psq")
                psv = p1ps.tile([128, JC, 128], dt.float32, tag="psv")
                for j in range(JC):
                    lhs = xt[:, j * 128:(j + 1) * 128]
                    nc.tensor.matmul(psq[:, j, :], lhs, wq[:], start=True, stop=True)
                    nc.tensor.matmul(psv[:, j, :], lhs, wvt[:], start=True, stop=True)
                qv = p1.tile([128, JC, 256], dt.bfloat16, tag="qvt")
                nc.scalar.copy(out=qv[:, :, 0:128], in_=psq[:])
                nc.scalar.copy(out=qv[:, :, 128:256], in_=psv[:])
                nc.sync.dma_start(
                    out=qv_d[1 + t * CHUNK:1 + (t + 1) * CHUNK, :]
                    .rearrange("(p j) e -> p j e", j=JC),
                    in_=qv[:],
                )

        qv_lo = qv_d[0:LO_MAX + 2, :]
        qv_hi = qv_d[1 + HI_BASE:1 + NPAD, :]

        psum = ctx.enter_context(tc.tile_pool(name="ps", bufs=1, space="PSUM"))
        accps = ctx.enter_context(tc.tile_pool(name="accps", bufs=2, space="PSUM"))

        # deferred-LN collection buffers (persist across the block loop)
        y2a = consts.tile([128, NB, 128], dt.float32)
        mva = consts.tile([128, NB, 2], dt.float32)

        # ---------- phase 2: per dest-block ----------
        for b in range(NB if stage != "p1" else 0):
            rl, rh = RL[b], RH[b]
            rt = rl + rh
            # K for this block: K = xtd_b.T @ wk (scaled 1/sqrt(HD) on copy)
            xtd = kpool.tile([128, 128], dt.bfloat16, tag="xtd")
            nc.sync.dma_start(out=xtd[:], in_=xtd_d[:, b * 128:(b + 1) * 128])
            kps = psum.tile([128, 128], dt.float32, tag="kps")
            nc.tensor.matmul(kps[:], xtd[:], wk[:], start=True, stop=True)
            kd = kpool.tile([128, 128], dt.bfloat16, tag="kd")
            nc.vector.tensor_scalar_mul(kd[:], kps[:], 1.0 / math.sqrt(HD))

            # gathers (whole block)
            glo = gpool.tile([128, RLMAX, 256], dt.bfloat16, tag="glo")
            ghi = gpool.tile([128, RHMAX, 256], dt.bfloat16, tag="ghi")
            if rl:
                ilo = mpool.tile([128, RLMAX * 8], dt.int16, tag="ilo")
                nc.sync.dma_start(out=ilo[:, :rl * 8],
                                  in_=idxlo_d[:, lo_off[b] * 8:(lo_off[b] + rl) * 8])
                nc.gpsimd.dma_gather(glo[:, :rl, :], qv_lo, ilo[:, :rl * 8],
                                     rl * 128, rl * 128, 256, elem_step=256,
                                     single_packet=False, queue_num=b % 4)
            if rh:
                ihi = mpool.tile([128, RHMAX * 8], dt.int16, tag="ihi")
                nc.sync.dma_start(out=ihi[:, :rh * 8],
                                  in_=idxhi_d[:, hi_off[b] * 8:(hi_off[b] + rh) * 8])
                nc.gpsimd.dma_gather(ghi[:, :rh, :], qv_hi, ihi[:, :rh * 8],
                                     rh * 128, rh * 128, 256, elem_step=256,
                                     single_packet=False, queue_num=b % 4)

            if stage == "gather":
                yg = opool.tile([128, 128], dt.float32, tag="yg")
                nc.vector.tensor_copy(out=yg[:], in_=glo[:, 0, 0:128])
                nc.sync.dma_start(out=y_d[b * 128:(b + 1) * 128, :], in_=yg[:])
                continue

            # edge weights for the whole block
            eat = mpool.tile([128, RLMAX + RHMAX, 16], dt.float32, tag="eat")
            nc.sync.dma_start(out=eat[:, :rt, :],
                              in_=ea_d[:, ea_off[b] * 16:(ea_off[b] + rt) * 16]
                              .rearrange("p (r s) -> p r s", s=16))
            # ew = sigmoid(sum ea) = 1 / (1 + exp(-sum)); Exp keeps the ACT
            # engine on a single LUT (no Sigmoid table swaps)
            easum = spool.tile([128, RLMAX + RHMAX], dt.float32, tag="easum")
            nc.vector.tensor_reduce(easum[:, :rt], eat[:, :rt, :],
                                    axis=mybir.AxisListType.X, op=AL.add,
                                    negate=True)
            een = spool.tile([128, RLMAX + RHMAX], dt.float32, tag="een")
            nc.scalar.activation(out=een[:, :rt], in_=easum[:, :rt], func=AF.Exp)
            ew1 = spool.tile([128, RLMAX + RHMAX], dt.float32, tag="ew1")
            nc.vector.tensor_scalar_add(ew1[:, :rt], een[:, :rt], 1.0)
            ew = spool.tile([128, RLMAX + RHMAX], dt.float32, tag="ew")
            nc.vector.reciprocal(out=ew[:, :rt], in_=ew1[:, :rt])

            # accumulator in PSUM via identity-matmul accumulation
            acc = accps.tile([128, 128], dt.float32, tag="acc")

            first_mm = True
            # chunks: lo rounds then hi rounds
            segs = []
            r0 = 0
            while r0 < rl:
                c = min(RCHUNK, rl - r0)
                segs.append((glo, r0, r0, c))
                r0 += c
            r0 = 0
            while r0 < rh:
                c = min(RCHUNK, rh - r0)
                segs.append((ghi, r0, rl + r0, c))
                r0 += c
            n_mm = sum(c for (_, _, _, c) in segs)
            mm_i = 0
            for (gt, gr, ar, c) in segs:
                g = gt[:, gr:gr + c, :]
                # prod = Qg * K (bcast over rounds)  [128, c, 128] bf16
                prod = cpool.tile([128, RCHUNK, 128], dt.bfloat16, tag="prod")
                kb = bass.AP(tensor=kd.tensor, offset=kd.offset,
                             ap=[list(kd.ap[0]), [0, c], [1, 128]])
                nc.vector.tensor_tensor(out=prod[:, :c, :], in0=g[:, :, 0:128],
                                        in1=kb, op=AL.mult)
                # head reduce via pairwise tree (tensor_reduce is 1x; dense
                # bf16 adds run 2x) -> [128, c, 8] f32
                p4 = prod[:, :c, :].rearrange("p c (h s) -> p c h s", s=16)
                t1_ = cpool.tile([128, RCHUNK, 8, 8], dt.bfloat16, tag="tr1")
                nc.vector.tensor_tensor(out=t1_[:, :c, :, :], in0=p4[:, :, :, 0:8],
                                        in1=p4[:, :, :, 8:16], op=AL.add)
                t2_ = cpool.tile([128, RCHUNK, 8, 4], dt.bfloat16, tag="tr2")
                nc.vector.tensor_tensor(out=t2_[:, :c, :, :], in0=t1_[:, :c, :, 0:4],
                                        in1=t1_[:, :c, :, 4:8], op=AL.add)
                t3_ = cpool.tile([128, RCHUNK, 8, 2], dt.bfloat16, tag="tr3")
                nc.vector.tensor_tensor(out=t3_[:, :c, :, :], in0=t2_[:, :c, :, 0:2],
                                        in1=t2_[:, :c, :, 2:4], op=AL.add)
                sraw = spool.tile([128, RCHUNK, 8], dt.float32, tag="sraw")
                nc.vector.tensor_tensor(out=sraw[:, :c, :], in0=t3_[:, :c, :, 0],
                                        in1=t3_[:, :c, :, 1], op=AL.add)
                # leaky relu: max(alpha*x, x)
                slr = spool.tile([128, RCHUNK, 8], dt.float32, tag="slr")
                nc.vector.scalar_tensor_tensor(out=slr[:, :c, :], in0=sraw[:, :c, :],
                                               scalar=ALPHA, in1=sraw[:, :c, :],
                                               op0=AL.mult, op1=AL.max)
                # * edge weight (bcast over heads)
                ewb = bass.AP(tensor=ew.tensor, offset=ew.offset + ar,
                              ap=[list(ew.ap[0]), [1, c], [0, 8]])
                sw = spool.tile([128, RCHUNK, 8], dt.float32, tag="sw")
                nc.vector.tensor_tensor(out=sw[:, :c, :], in0=slr[:, :c, :],
                                        in1=ewb, op=AL.mult)
                # exp (scores are small; no max-sub needed)
                esc = spool.tile([128, RCHUNK, 8], dt.float32, tag="esc")
                nc.scalar.activation(out=esc[:, :c, :], in_=sw[:, :c, :], func=AF.Exp)
                # sum over heads + reciprocal
                ses = spool.tile([128, RCHUNK], dt.float32, tag="ses")
                nc.vector.tensor_reduce(ses[:, :c], esc[:, :c, :],
                                        axis=mybir.AxisListType.X, op=AL.add)
                rec = spool.tile([128, RCHUNK], dt.float32, tag="rec")
                nc.vector.reciprocal(out=rec[:, :c], in_=ses[:, :c])
                # probs = esc * rec (bcast over heads) -> bf16
                rcb = bass.AP(tensor=rec.tensor, offset=rec.offset,
                              ap=[list(rec.ap[0]), [1, c], [0, 8]])
                probs = spool.tile([128, RCHUNK, 8], dt.bfloat16, tag="probs")
                nc.vector.tensor_tensor(out=probs[:, :c, :], in0=esc[:, :c, :],
                                        in1=rcb, op=AL.mult)
                # wv = Vg * probs (bcast 16 within head) [128, c, 128] bf16
                pb = bass.AP(tensor=probs.tensor, offset=probs.offset,
                             ap=[list(probs.ap[0]), [8, c], [1, 8], [0, 16]])
                wvt_t = cpool.tile([128, RCHUNK, 128], dt.bfloat16, tag="wv")
                nc.vector.tensor_tensor(out=wvt_t[:, :c, :], in0=g[:, :, 128:256],
                                        in1=pb, op=AL.mult)
                # accumulate: acc += I.T @ wv_r  (PE identity accumulation)
                for r in range(c):
                    nc.tensor.matmul(acc[:], ident[:], wvt_t[:, r, :],
                                     start=(mm_i == 0), stop=(mm_i == n_mm - 1),
                                     skip_group_check=True)
                    mm_i += 1

            if stage == "compute":
                yg = opool.tile([128, 128], dt.float32, tag="yg")
                nc.vector.tensor_copy(out=yg[:], in_=acc[:])
                nc.sync.dma_start(out=y_d[b * 128:(b + 1) * 128, :], in_=yg[:])
                continue

            # ---------- output stage (LN sqrt deferred + batched) ----------
            accs = opool.tile([128, 128], dt.bfloat16, tag="accs")
            nc.vector.tensor_copy(out=accs[:], in_=acc[:])
            accT = psum.tile([128, 128], dt.bfloat16, tag="accT")
            nc.tensor.transpose(accT[:], accs[:], ident[:])
            accTs = opool.tile([128, 128], dt.bfloat16, tag="accTs")
            nc.vector.tensor_copy(out=accTs[:], in_=accT[:])
            oproj = psum.tile([128, 128], dt.float32, tag="oproj")
            nc.tensor.matmul(oproj[:], accTs[:], wo[:], start=True, stop=True)

            xdt = opool.tile([128, 128], dt.float32, tag="xdt")
            nc.sync.dma_start(out=xdt[:], in_=xd_d[b * 128:(b + 1) * 128, :])
            y1 = opool.tile([128, 128], dt.float32, tag="y1")
            nc.vector.tensor_tensor(out=y1[:], in0=oproj[:], in1=xdt[:], op=AL.add)
            nc.vector.tensor_tensor(out=y2a[:, b, :], in0=y1[:], in1=bo[:],
                                    op=AL.add)
            st = spool.tile([128, 6], dt.float32, tag="st")
            nc.vector.bn_stats(out=st[:], in_=y2a[:, b, :])
            nc.vector.bn_aggr(out=mva[:, b, :], in_=st[:])

        if stage == "full":
            # batched LN: one sqrt + reciprocal for all blocks
            sd = consts.tile([128, NB], dt.float32)
            nc.scalar.activation(out=sd[:], in_=mva[:, :, 1], func=AF.Sqrt,
                                 bias=epsT[:])
            rstd = consts.tile([128, NB], dt.float32)
            nc.vector.reciprocal(out=rstd[:], in_=sd[:])
            for b in range(NB):
                t1 = opool.tile([128, 128], dt.float32, tag="t1")
                nc.vector.scalar_tensor_tensor(out=t1[:], in0=y2a[:, b, :],
                                               scalar=mva[:, b, 0:1], in1=ga[:],
                                               op0=AL.subtract, op1=AL.mult)
                yn = opool.tile([128, 128], dt.float32, tag="yn")
                nc.vector.scalar_tensor_tensor(out=yn[:], in0=t1[:],
                                               scalar=rstd[:, b:b + 1], in1=be[:],
                                               op0=AL.mult, op1=AL.add)
                nc.sync.dma_start(out=y_d[b * 128:(b + 1) * 128, :], in_=yn[:])

    nc.compile()
    return nc


# ------------------------------------------------------------------ runner
def _in_maps(meta, arrs, w):
    NC = meta["cfg"]["NC"]
    maps = []
    for c in range(NC):
        maps.append(dict(
            xT=np.ascontiguousarray(arrs["xT"]),
            xtd=np.ascontiguousarray(arrs["xtd"][c]),
            xd=np.ascontiguousarray(arrs["xd"][c]),
            idx_lo=np.ascontiguousarray(arrs["idx_lo"][c]),
            idx_hi=np.ascontiguousarray(arrs["idx_hi"][c]),
            ea=np.ascontiguousarray(arrs["ea"][c]),
            **{k: np.ascontiguousarray(v) for k, v in w.items()},
        ))
    return maps


def assemble(meta, arrs, results):
    cfg = meta["cfg"]
    N, NC, DPC = cfg["N"], cfg["NC"], meta["DPC"]
    out = np.empty((N, D), dtype=np.float32)
    for c in range(NC):
        yc = results[c]["y"]
        perm = arrs["perms"][c]
        valid = perm >= 0
        out[c * DPC + perm[valid]] = yc[:meta["DPAD"]][valid]
    return out


_CACHE = {}


def kernel(x, edge_index, edge_attr, Wq, Wk, Wv, Wo, bo, gamma, beta):
    cfg = FULL_CFG
    meta, arrs = host_prep(x, edge_index, edge_attr, cfg)
    w = host_weights(Wq, Wk, Wv, Wo, bo, gamma, beta)
    key = (tuple(meta["RL"]), tuple(meta["RH"]))
    if key not in _CACHE:
        _CACHE[key] = build_nc(meta)
    nc = _CACHE[key]
    from concourse.bass_utils import run_bass_kernel_spmd
    res = run_bass_kernel_spmd(nc, _in_maps(meta, arrs, w),
                               core_ids=list(range(cfg["NC"])))
    return assemble(meta, arrs, res.results)


if __name__ == "__main__":
    import reference
    inputs = {k: np.asarray(v) for k, v in reference.setup_inputs().items()}
    out = kernel(**inputs)
    exp = np.asarray(reference.reference(**reference.setup_inputs()))
    err = np.abs(out - exp).max() / max(np.abs(exp).max(), 1e-9)
    print("Relative error:", err)



# revision 7
# speedup vs baseline: 2.8751x; 2.8751x over previous
"""CrossDomainGAT Trainium2 kernel — gather-free streaming design.

Strategy (graph/data parallel, per sharding hint):
  - Destination nodes sharded across 8 cores (6250 dests/core, padded to 6272 =
    49 blocks x 128).  Edges routed to the core owning the destination, so the
    per-edge softmax (over heads -- edge-local) and the scatter-add stay local.
  - The previous design gathered Q|V rows per edge with gpsimd.dma_gather;
    SWDGE descriptor generation (~6.5 ns/row on the Pool Q7) made GpSimd the
    bottleneck (~780 us/core).  Instead the HOST routes each edge's source-x
    row into a dense fp8 stream in slot order (a pure permutation/duplication,
    no arithmetic), and the DEVICE projects Q|V per edge on the TensorEngine:
        per round r: matmul(lhsT = xgT[:, r*128:(r+1)*128] (fp8),
                            rhs  = [64*Wq^T | 64*Wv^T(perm)] (fp8)) -> PSUM
    This reads 128 B/edge (vs 512 B gathered) sequentially at full DMA
    bandwidth and costs zero descriptor-generation time.
  - fp8 weights are pre-scaled by 64 (dodges e4m3 denormals at |w|~0.02); the
    scale is compensated exactly: K is scaled by 1/(sqrt(HD)*64*4... see kd),
    and Wo by 1/64 on the host (powers of two, exact).
  - The V half of the projection output is feature-interleaved h-fastest
    (column j*8+h holds true feature h*16+j) so the probs broadcast in
    wv = V * probs has a stride-1 innermost AP and every big DVE op runs in
    2x (16-bit dual-pump) mode.  Wo rows are permuted to match.
  - Dest blocks are grouped (<= 4 blocks, <= GCMAX rounds, uniform rounds per
    block within a group) so DVE/ACT ops batch over the whole group.
  - Scatter-add accumulation runs on the TensorEngine as identity-matmul
    accumulation into PSUM (per dest block); output projection + residual +
    LayerNorm with the sqrt deferred and batched across blocks.
"""

import math
import numpy as np
import ml_dtypes

# ---------------------------------------------------------------- problem cfg
D = 128
H = 8
HD = 16
ALPHA = 0.2
LN_EPS = 1e-5
WSCALE = 64.0          # fp8 weight pre-scale (power of 2)

FULL_CFG = dict(
    N=50000,
    E=800000,
    NC=8,          # cores
    GMAX=4,        # max dest blocks per group
    GCMAX=56,      # max rounds per group (SBUF budget)
    CC=4,          # rounds per PSUM chunk (matmul -> ACT copy granularity)
)

FP8 = ml_dtypes.float8_e4m3  # TRN fp8_e4m3 (IEEE-ish, max 240) byte-compatible


def _hperm():
    """V-feature permutation: position j*8+h <- true feature h*16+j."""
    pos = np.arange(128)
    j, h = pos // 8, pos % 8
    return h * 16 + j          # true feature index for each position


def host_prep(x, edge_index, edge_attr, cfg):
    """Route edges per core, build the per-edge x stream + block metadata."""
    N, E, NC = cfg["N"], cfg["E"], cfg["NC"]
    GMAX, GCMAX = cfg["GMAX"], cfg["GCMAX"]
    DPC = N // NC                      # dests per core
    NB = (DPC + 127) // 128            # dest blocks per core
    DPAD = NB * 128

    row = np.asarray(edge_index[0], dtype=np.int64)
    col = np.asarray(edge_index[1], dtype=np.int64)
    ea = np.asarray(edge_attr, dtype=np.float32)
    x = np.asarray(x, dtype=np.float32)

    core = col // DPC
    cl = col - core * DPC              # local dest id

    # ---- per-core degree sort; R[b] = max degree in block, maxed over cores
    per_core = []
    Rc = np.zeros((NC, NB), dtype=np.int64)
    for c in range(NC):
        m = core == c
        clc = cl[m]
        deg = np.bincount(clc, minlength=DPC)
        order = np.argsort(deg, kind="stable")          # ascending degree
        perm = np.concatenate([order, np.full(DPAD - DPC, -1, np.int64)])
        inv = np.empty(DPC, dtype=np.int64)
        inv[order] = np.arange(DPC)
        q = inv[clc]                                    # dest slot of each edge
        degs = np.concatenate([deg[order], np.zeros(DPAD - DPC, np.int64)])
        Rc[c] = np.maximum(degs.reshape(NB, 128).max(axis=1), 1)
        per_core.append(dict(m=m, q=q, perm=perm))

    R = Rc.max(axis=0)                 # uniform across cores (SPMD)

    # ---- group blocks: <= GMAX blocks, uniform rounds Rg = max R in group,
    # total rounds G*Rg <= GCMAX (ascending R makes the padding tiny)
    groups = []                        # (b0, G, Rg)
    b0 = 0
    while b0 < NB:
        G = 1
        while (b0 + G < NB and G < GMAX
               and (G + 1) * max(R[b0:b0 + G + 1]) <= GCMAX):
            G += 1
        groups.append((b0, G, int(max(R[b0:b0 + G]))))
        b0 += G
    # round offset of each block
    blk_off = np.zeros(NB, dtype=np.int64)
    off = 0
    for (b0, G, Rg) in groups:
        for k in range(G):
            blk_off[b0 + k] = off + k * Rg
        off += G * Rg
    n_rounds = int(off)
    S = n_rounds * 128                 # stream slots per core

    hp = _hperm()

    xg_arrs, ea_arrs, xd_arrs, xtd_arrs, perms = [], [], [], [], []
    xpad = np.concatenate([x, np.zeros((1, D), np.float32)])   # -1 -> zero row
    for c in range(NC):
        pc = per_core[c]
        m, q, perm = pc["m"], pc["q"], pc["perm"]
        p = q % 128
        b = q // 128
        # rank within dest
        sort = np.argsort(q, kind="stable")
        qs = q[sort]
        starts = np.r_[0, np.flatnonzero(np.diff(qs)) + 1]
        counts = np.diff(np.r_[starts, len(qs)])
        rank_sorted = np.arange(len(qs)) - np.repeat(starts, counts)
        rank = np.empty(len(qs), np.int64)
        rank[sort] = rank_sorted

        rr = blk_off[b] + rank                          # absolute round
        src_slot = np.full((n_rounds, 128), -1, dtype=np.int64)
        src_slot[rr, p] = row[m]
        # stream: [128 xf, n_rounds*128] fp8
        xg3 = xpad[src_slot.reshape(-1)]                # [S, 128] f32
        xgT = np.ascontiguousarray(xg3.T).astype(FP8)   # [128, S]
        xg_arrs.append(xgT)

        # edge attrs: [128 p, n_rounds, 16] bf16
        eac = np.zeros((128, n_rounds, 16), dtype=np.float32)
        eac[p, rr] = ea[m]
        ea_arrs.append(eac.reshape(128, -1).astype(ml_dtypes.bfloat16))

        # dest-side x (residual) and xT (K build), permuted to slot order
        xd = np.zeros((DPAD, D), dtype=np.float32)
        valid = perm >= 0
        xd[valid] = x[c * DPC + perm[valid]]
        xd_arrs.append(xd)
        xtd_arrs.append(np.ascontiguousarray(xd.T).astype(ml_dtypes.bfloat16))
        perms.append(perm)

    meta = dict(
        cfg=cfg, DPC=DPC, NB=NB, DPAD=DPAD,
        R=R.astype(int).tolist(), groups=groups,
        blk_off=blk_off.astype(int).tolist(), n_rounds=n_rounds, S=S,
    )
    arrs = dict(xg=xg_arrs, ea=ea_arrs, xd=xd_arrs, xtd=xtd_arrs, perms=perms)
    return meta, arrs


# ------------------------------------------------------------------ weights
def host_weights(Wq, Wk, Wv, Wo, bo, gamma, beta):
    bf = ml_dtypes.bfloat16
    hp = _hperm()
    Wq = np.asarray(Wq, np.float32)
    Wk = np.asarray(Wk, np.float32)
    Wv = np.asarray(Wv, np.float32)
    Wo = np.asarray(Wo, np.float32)
    # wqv: [128 xf, 256] fp8 = [64*Wq^T | 64*Wv^T with V-cols permuted]
    wqv = np.empty((128, 256), np.float32)
    wqv[:, 0:128] = Wq.T * WSCALE
    wqv[:, 128:256] = (Wv.T * WSCALE)[:, hp]
    wqv = np.clip(wqv, -240, 240).astype(FP8)
    # wo: rows permuted to match the V interleave; scaled 1/WSCALE
    wo_t = np.ascontiguousarray((Wo.T / WSCALE)[hp, :]).astype(bf)
    rep = lambda v: np.tile(np.asarray(v, np.float32)[None, :], (128, 1))
    return dict(
        wqv=wqv,
        wk_t=np.ascontiguousarray(Wk.T).astype(bf),
        wo_t=wo_t,
        bo_b=rep(bo), gamma_b=rep(gamma), beta_b=rep(beta),
        ident=np.eye(128, dtype=np.float32).astype(bf),
    )


# ------------------------------------------------------------------ kernel IR
def build_nc(meta, debug=False, stage=None):
    import os as _os
    stage = stage or _os.environ.get("K_STAGE", "full")
    from contextlib import ExitStack
    import concourse.bacc as bacc
    import concourse.bass as bass
    import concourse.tile as tile
    from concourse import mybir

    cfg = meta["cfg"]
    NB, DPAD = meta["NB"], meta["DPAD"]
    R, groups, blk_off = meta["R"], meta["groups"], meta["blk_off"]
    n_rounds, S = meta["n_rounds"], meta["S"]
    CC = cfg["CC"]
    GM = cfg["GMAX"]
    CMAX = max(G * Rg for (_, G, Rg) in groups)

    dt = mybir.dt
    AF = mybir.ActivationFunctionType
    AL = mybir.AluOpType

    nc = bacc.Bacc("TRN2", target_bir_lowering=False, debug=debug)

    # ---------- I/O ----------
    xg_d = nc.dram_tensor("xg", [128, S], dt.float8e4, kind="ExternalInput")
    ea_d = nc.dram_tensor("ea", [128, n_rounds * 16], dt.bfloat16,
                          kind="ExternalInput")
    xtd_d = nc.dram_tensor("xtd", [128, DPAD], dt.bfloat16, kind="ExternalInput")
    xd_d = nc.dram_tensor("xd", [DPAD, 128], dt.float32, kind="ExternalInput")
    wqv_d = nc.dram_tensor("wqv", [128, 256], dt.float8e4, kind="ExternalInput")
    wk_d = nc.dram_tensor("wk_t", [128, 128], dt.bfloat16, kind="ExternalInput")
    wo_d = nc.dram_tensor("wo_t", [128, 128], dt.bfloat16, kind="ExternalInput")
    bo_d = nc.dram_tensor("bo_b", [128, 128], dt.float32, kind="ExternalInput")
    ga_d = nc.dram_tensor("gamma_b", [128, 128], dt.float32, kind="ExternalInput")
    be_d = nc.dram_tensor("beta_b", [128, 128], dt.float32, kind="ExternalInput")
    id_d = nc.dram_tensor("ident", [128, 128], dt.bfloat16, kind="ExternalInput")
    y_d = nc.dram_tensor("y", [DPAD, 128], dt.float32, kind="ExternalOutput")

    KS = 1.0 / (math.sqrt(HD) * WSCALE * WSCALE)  # kd scale: prod = Q64*K/256/4

    with tile.TileContext(nc) as tc, ExitStack() as ctx:
        consts = ctx.enter_context(tc.tile_pool(name="consts", bufs=1))
        xpool = ctx.enter_context(tc.tile_pool(name="xg", bufs=2))
        qpool = ctx.enter_context(tc.tile_pool(name="qv", bufs=2))
        wpool = ctx.enter_context(tc.tile_pool(name="wv", bufs=2))
        ppool = ctx.enter_context(tc.tile_pool(name="prod", bufs=1))
        epool = ctx.enter_context(tc.tile_pool(name="ea", bufs=2))
        spool = ctx.enter_context(tc.tile_pool(name="small", bufs=2))
        kpool = ctx.enter_context(tc.tile_pool(name="kblk", bufs=2))
        opool = ctx.enter_context(tc.tile_pool(name="outs", bufs=3))
        psqv_ps = ctx.enter_context(tc.tile_pool(name="psqv", bufs=2, space="PSUM"))
        misc_ps = ctx.enter_context(tc.tile_pool(name="miscps", bufs=1, space="PSUM"))
        acc_ps = ctx.enter_context(tc.tile_pool(name="accps", bufs=2, space="PSUM"))

        # ---------- constants ----------
        wqv = consts.tile([128, 256], dt.float8e4)
        wk = consts.tile([128, 128], dt.bfloat16)
        wo = consts.tile([128, 128], dt.bfloat16)
        bo = consts.tile([128, 128], dt.float32)
        ga = consts.tile([128, 128], dt.float32)
        be = consts.tile([128, 128], dt.float32)
        ident = consts.tile([128, 128], dt.bfloat16)
        epsT = consts.tile([128, 1], dt.float32)
        for dst, src in ((wqv, wqv_d), (wk, wk_d), (wo, wo_d), (bo, bo_d),
                         (ga, ga_d), (be, be_d), (ident, id_d)):
            nc.sync.dma_start(out=dst[:], in_=src[:])
        nc.vector.memset(epsT[:], LN_EPS)

        # deferred-LN collection buffers (persist across the group loop)
        y2a = consts.tile([128, NB, 128], dt.float32)
        mva = consts.tile([128, NB, 2], dt.float32)

        # ---------- main loop over groups ----------
        for (b0, G, Rg) in groups:
            c = G * Rg                              # rounds in this group
            off = blk_off[b0]                       # absolute first round
            # stream + edge-attr + dest-side DMAs
            xg = xpool.tile([128, CMAX * 128], dt.float8e4, tag="xg")
            nc.sync.dma_start(out=xg[:, :c * 128],
                              in_=xg_d[:, off * 128:(off + c) * 128])
            eat = epool.tile([128, CMAX, 16], dt.bfloat16, tag="eat")
            nc.sync.dma_start(out=eat[:, :c, :],
                              in_=ea_d[:, off * 16:(off + c) * 16]
                              .rearrange("p (r s) -> p r s", s=16))
            xtd = kpool.tile([128, GM * 128], dt.bfloat16, tag="xtd")
            nc.sync.dma_start(out=xtd[:, :G * 128],
                              in_=xtd_d[:, b0 * 128:(b0 + G) * 128])

            # K for the group's blocks: kps[k] = xtd_k.T @ wk
            # (kps borrows a psqv-pool buffer; it is consumed into kd before
            # the pool rotates back to this buffer)
            kpt = psqv_ps.tile([128, CC, 256], dt.float32, tag="ps")
            kview = lambda o, n: bass.AP(tensor=kpt.tensor, offset=kpt.offset + o,
                                         ap=[list(kpt.ap[0]), [1, n]])
            for k in range(G):
                nc.tensor.matmul(kview(k * 128, 128),
                                 xtd[:, k * 128:(k + 1) * 128],
                                 wk[:], start=True, stop=True)
            kd = kpool.tile([128, GM * 128], dt.bfloat16, tag="kd")
            nc.vector.tensor_scalar_mul(kd[:, :G * 128], kview(0, G * 128), KS)

            # per-edge Q|V projection: per round one matmul, ACT copies CC
            # rounds per chunk from PSUM to bf16 SBUF
            qv = qpool.tile([128, CMAX, 256], dt.bfloat16, tag="qv")
            r0 = 0
            while r0 < c:
                cc = min(CC, c - r0)
                ps = psqv_ps.tile([128, CC, 256], dt.float32, tag="ps")
                for j in range(cc):
                    nc.tensor.matmul(ps[:, j, :],
                                     xg[:, (r0 + j) * 128:(r0 + j + 1) * 128],
                                     wqv[:], start=True, stop=True)
                nc.scalar.copy(out=qv[:, r0:r0 + cc, :], in_=ps[:, :cc, :])
                r0 += cc

            if stage == "stream":
                yg = opool.tile([128, 128], dt.float32, tag="yg")
                nc.vector.tensor_copy(out=yg[:], in_=qv[:, 0, 0:128])
                nc.sync.dma_start(out=y_d[b0 * 128:(b0 + 1) * 128, :], in_=yg[:])
                continue

            # ---------- edge math, batched over the whole group ----------
            # ew = sigmoid(sum ea) = 1/(1+exp(-sum))
            easum = spool.tile([128, CMAX], dt.float32, tag="easum")
            nc.vector.tensor_reduce(easum[:, :c], eat[:, :c, :],
                                    axis=mybir.AxisListType.X, op=AL.add,
                                    negate=True)
            een = spool.tile([128, CMAX], dt.float32, tag="een")
            nc.scalar.activation(out=een[:, :c], in_=easum[:, :c], func=AF.Exp)
            ew1 = spool.tile([128, CMAX], dt.float32, tag="ew1")
            nc.vector.tensor_scalar_add(ew1[:, :c], een[:, :c], 1.0)
            ew = spool.tile([128, CMAX], dt.float32, tag="ew")
            nc.vector.reciprocal(out=ew[:, :c], in_=ew1[:, :c])

            # prod = Q64 * kd (bcast per block over Rg rounds)
            prod = ppool.tile([128, CMAX, 128], dt.bfloat16, tag="prod")
            kb = bass.AP(tensor=kd.tensor, offset=kd.offset,
                         ap=[list(kd.ap[0]), [128, G], [0, Rg], [1, 128]])
            nc.vector.tensor_tensor(out=prod[:, :c, :], in0=qv[:, :c, 0:128],
                                    in1=kb, op=AL.mult)
            # head reduce via pairwise tree (bf16 2x; tensor_reduce is 1x)
            p4 = prod[:, :c, :].rearrange("p c (h s) -> p c h s", s=16)
            t1_ = ppool.tile([128, CMAX, 8, 8], dt.bfloat16, tag="tr1")
            nc.vector.tensor_tensor(out=t1_[:, :c, :, :], in0=p4[:, :, :, 0:8],
                                    in1=p4[:, :, :, 8:16], op=AL.add)
            t2_ = ppool.tile([128, CMAX, 8, 4], dt.bfloat16, tag="tr2")
            nc.vector.tensor_tensor(out=t2_[:, :c, :, :], in0=t1_[:, :c, :, 0:4],
                                    in1=t1_[:, :c, :, 4:8], op=AL.add)
            t3_ = ppool.tile([128, CMAX, 8, 2], dt.bfloat16, tag="tr3")
            nc.vector.tensor_tensor(out=t3_[:, :c, :, :], in0=t2_[:, :c, :, 0:2],
                                    in1=t2_[:, :c, :, 2:4], op=AL.add)
            sraw = spool.tile([128, CMAX, 8], dt.float32, tag="sraw")
            nc.vector.tensor_tensor(out=sraw[:, :c, :], in0=t3_[:, :c, :, 0],
                                    in1=t3_[:, :c, :, 1], op=AL.add)
            # leaky relu: max(alpha*x, x)
            slr = spool.tile([128, CMAX, 8], dt.float32, tag="slr")
            nc.vector.scalar_tensor_tensor(out=slr[:, :c, :], in0=sraw[:, :c, :],
                                           scalar=ALPHA, in1=sraw[:, :c, :],
                                           op0=AL.mult, op1=AL.max)
            # * edge weight (bcast over heads)
            ewb = bass.AP(tensor=ew.tensor, offset=ew.offset,
                          ap=[list(ew.ap[0]), [1, c], [0, 8]])
            sw = spool.tile([128, CMAX, 8], dt.float32, tag="sw")
            nc.vector.tensor_tensor(out=sw[:, :c, :], in0=slr[:, :c, :],
                                    in1=ewb, op=AL.mult)
            # exp (scores are small; no max-sub needed)
            esc = spool.tile([128, CMAX, 8], dt.float32, tag="esc")
            nc.scalar.activation(out=esc[:, :c, :], in_=sw[:, :c, :], func=AF.Exp)
            # sum over heads + reciprocal
            ses = spool.tile([128, CMAX], dt.float32, tag="ses")
            nc.vector.tensor_reduce(ses[:, :c], esc[:, :c, :],
                                    axis=mybir.AxisListType.X, op=AL.add)
            rec = spool.tile([128, CMAX], dt.float32, tag="rec")
            nc.vector.reciprocal(out=rec[:, :c], in_=ses[:, :c])
            # probs = esc * rec (bcast over heads) -> bf16
            rcb = bass.AP(tensor=rec.tensor, offset=rec.offset,
                          ap=[list(rec.ap[0]), [1, c], [0, 8]])
            probs = spool.tile([128, CMAX, 8], dt.bfloat16, tag="probs")
            nc.vector.tensor_tensor(out=probs[:, :c, :], in0=esc[:, :c, :],
                                    in1=rcb, op=AL.mult)
            # wv = V64 * probs (V is h-fastest interleaved: bcast [0,16],[1,8])
            pb = bass.AP(tensor=probs.tensor, offset=probs.offset,
                         ap=[list(probs.ap[0]), [8, c], [0, 16], [1, 8]])
            wvt = wpool.tile([128, CMAX, 128], dt.bfloat16, tag="wv")
            nc.vector.tensor_tensor(out=wvt[:, :c, :], in0=qv[:, :c, 128:256],
                                    in1=pb, op=AL.mult)

            # accumulate per block: acc_k += I.T @ wv_r  (PE PSUM accumulation)
            acc = acc_ps.tile([128, GM, 128], dt.float32, tag="acc")
            for k in range(G):
                for r in range(Rg):
                    nc.tensor.matmul(acc[:, k, :], ident[:],
                                     wvt[:, k * Rg + r, :],
                                     start=(r == 0), stop=(r == Rg - 1),
                                     skip_group_check=True)

            # ---------- output stage (batched per group; LN sqrt deferred) ---
            xdt = opool.tile([128, GM, 128], dt.float32, tag="xdt")
            nc.sync.dma_start(
                out=xdt[:, :G, :],
                in_=xd_d[b0 * 128:(b0 + G) * 128, :]
                .rearrange("(g p) e -> p g e", g=G))
            accs = opool.tile([128, GM * 128], dt.bfloat16, tag="accs")
            nc.vector.tensor_copy(out=accs[:, :G * 128], in_=acc[:, :G, :])
            accT = misc_ps.tile([128, GM * 128], dt.bfloat16, tag="accT")
            for k in range(G):
                nc.tensor.transpose(accT[:, k * 128:(k + 1) * 128],
                                    accs[:, k * 128:(k + 1) * 128], ident[:])
            accTs = opool.tile([128, GM * 128], dt.bfloat16, tag="accTs")
            nc.vector.tensor_copy(out=accTs[:, :G * 128], in_=accT[:, :G * 128])
            oproj = misc_ps.tile([128, GM, 128], dt.float32, tag="oproj")
            for k in range(G):
                nc.tensor.matmul(oproj[:, k, :],
                                 accTs[:, k * 128:(k + 1) * 128],
                                 wo[:], start=True, stop=True)
            y1 = opool.tile([128, GM, 128], dt.float32, tag="y1")
            nc.vector.tensor_tensor(out=y1[:, :G, :], in0=oproj[:, :G, :],
                                    in1=xdt[:, :G, :], op=AL.add)
            bob = bass.AP(tensor=bo.tensor, offset=bo.offset,
                          ap=[list(bo.ap[0]), [0, G], [1, 128]])
            nc.vector.tensor_tensor(out=y2a[:, b0:b0 + G, :], in0=y1[:, :G, :],
                                    in1=bob, op=AL.add)
            for k in range(G):
                b = b0 + k
                st = spool.tile([128, 6], dt.float32, tag="st")
                nc.vector.bn_stats(out=st[:], in_=y2a[:, b, :])
                nc.vector.bn_aggr(out=mva[:, b, :], in_=st[:])

        if stage == "full":
            # batched LN: one sqrt + reciprocal for all blocks
            sd = consts.tile([128, NB], dt.float32)
            nc.scalar.activation(out=sd[:], in_=mva[:, :, 1], func=AF.Sqrt,
                                 bias=epsT[:])
            rstd = consts.tile([128, NB], dt.float32)
            nc.vector.reciprocal(out=rstd[:], in_=sd[:])
            for b in range(NB):
                t1 = opool.tile([128, 128], dt.float32, tag="t1")
                nc.vector.scalar_tensor_tensor(out=t1[:], in0=y2a[:, b, :],
                                               scalar=mva[:, b, 0:1], in1=ga[:],
                                               op0=AL.subtract, op1=AL.mult)
                yn = opool.tile([128, 128], dt.float32, tag="yn")
                nc.vector.scalar_tensor_tensor(out=yn[:], in0=t1[:],
                                               scalar=rstd[:, b:b + 1], in1=be[:],
                                               op0=AL.mult, op1=AL.add)
                nc.sync.dma_start(out=y_d[b * 128:(b + 1) * 128, :], in_=yn[:])

    nc.compile()
    return nc


# ------------------------------------------------------------------ runner
def _in_maps(meta, arrs, w):
    NC = meta["cfg"]["NC"]
    maps = []
    for c in range(NC):
        maps.append(dict(
            xg=np.ascontiguousarray(arrs["xg"][c]),
            ea=np.ascontiguousarray(arrs["ea"][c]),
            xtd=np.ascontiguousarray(arrs["xtd"][c]),
            xd=np.ascontiguousarray(arrs["xd"][c]),
            **{k: np.ascontiguousarray(v) for k, v in w.items()},
        ))
    return maps


def assemble(meta, arrs, results):
    cfg = meta["cfg"]
    N, NC, DPC = cfg["N"], cfg["NC"], meta["DPC"]
    out = np.empty((N, D), dtype=np.float32)
    for c in range(NC):
        yc = results[c]["y"]
        perm = arrs["perms"][c]
        valid = perm >= 0
        out[c * DPC + perm[valid]] = yc[:meta["DPAD"]][valid]
    return out


_CACHE = {}


def kernel(x, edge_index, edge_attr, Wq, Wk, Wv, Wo, bo, gamma, beta):
    cfg = FULL_CFG
    meta, arrs = host_prep(x, edge_index, edge_attr, cfg)
    w = host_weights(Wq, Wk, Wv, Wo, bo, gamma, beta)
    key = (tuple(meta["R"]), tuple(meta["groups"]))
    if key not in _CACHE:
        _CACHE[key] = build_nc(meta)
    nc = _CACHE[key]
    from concourse.bass_utils import run_bass_kernel_spmd
    res = run_bass_kernel_spmd(nc, _in_maps(meta, arrs, w),
                               core_ids=list(range(cfg["NC"])))
    return assemble(meta, arrs, res.results)


if __name__ == "__main__":
    import reference
    inputs = {k: np.asarray(v) for k, v in reference.setup_inputs().items()}
    out = kernel(**inputs)
    exp = np.asarray(reference.reference(**reference.setup_inputs()))
    err = np.abs(out - exp).max() / max(np.abs(exp).max(), 1e-9)
    print("Relative error:", err)
